# revision 5
# baseline (speedup 1.0000x reference)
"""Trainium2 Bass kernel for multi-head attention (B=2, S=2048, D=1024, H=16).

Sharding: data-parallel over query rows. Core c handles batch b=c//4 and query
rows [512*(c%4), 512*(c%4+1)). Each core computes K/V projections for all 16
heads over the (compacted) key sequence, Q projection for its 512 rows,
attention, and the output projection for its rows. No cross-core collectives
(measured AllToAll on this fabric is ~2 ms for 2 MB — far too slow to use).

Key optimizations over the fp32r baseline:
  * bf16 operands everywhere (matmul rate is unchanged vs fp32r, but DMA and
    SBUF halve and DVE ops get cheaper); accumulation stays fp32 in PSUM.
  * Host-side key compaction: keys are permuted so valid (non-pad) keys come
    first (attention is permutation-invariant over keys), and the program is
    JIT-specialized on NC2 = the number of 128-key chunks that contain any
    valid key. With a ~50% pad mask this skips almost half of the K/V
    projection, score, exp and attn@V work. Masked keys inside the processed
    range are handled exactly as before: their V rows and denominator column
    are zeroed, so they contribute 0 to numerator and denominator.
  * Row-tiled score pairs: per group (2 heads), per s-chunk, two K=64 matmuls
    are issued at PE tile positions (0,0) and (64,0) (auto-derived from the
    operands' base partitions). The two heads' scores compute concurrently in
    the two halves of the PE array -- no zero-padded q needed.
  * V-projection bias is folded into the PSUM accumulation via a K=1
    ones-row x bias-row matmul, saving a DVE pass.
  * Elementwise work is split between DVE and GPSIMD so neither blocks PE/ACT.

Softmax skips max-subtraction (scores are ~N(0,1) after the 1/8 scale; exp
cannot overflow), so the denominator is a plain sum, obtained from the extra
65th column of the V tiles (matmul row 64).
"""

import math
import os
import sys

sys.path.insert(0, "/opt/trn_rl_repo")

import numpy as np

B, S, D, H, DH = 2, 2048, 1024, 16, 64
NCORES = 8
CPB = NCORES // B       # cores per batch
QB = S // CPB           # 512 query rows per core
P = 128
DCH = D // P            # 8 contraction chunks
SC = S // P             # 16 s-chunks max
HG = H // 2             # 8 head groups (2 heads each)

_compiled = {}
LAST_RESULTS = None
UNROLL = 1       # debug: repeat the whole body N times inside one NEFF
NC2 = SC         # key chunks to process (even, <= SC); set by prep_inputs


def _build_program():
    import concourse.bass as bass
    import concourse.mybir as mybir
    import concourse.tile as tile
    from concourse import bacc

    f32 = mybir.dt.float32
    bf16 = mybir.dt.bfloat16
    AF = mybir.ActivationFunctionType
    OP = mybir.AluOpType

    nc2 = NC2
    npc = nc2 // 2       # score/attnV chunk pairs
    SK = nc2 * P         # processed key length

    nc = bacc.Bacc(
        "TRN2", target_bir_lowering=False, debug=False,
        num_devices=NCORES,
    )

    xq = nc.dram_tensor("xq", [DCH, P, QB], bf16, kind="ExternalInput")
    xk = nc.dram_tensor("xk", [DCH, P, SK], bf16, kind="ExternalInput")
    wq = nc.dram_tensor("wq", [HG, P, DCH, P], bf16, kind="ExternalInput")
    wk = nc.dram_tensor("wk", [HG, P, DCH, P], bf16, kind="ExternalInput")
    wv = nc.dram_tensor("wv", [H // 4, P, DCH, 256], bf16, kind="ExternalInput")
    woT = nc.dram_tensor("woT", [DCH, P, D], bf16, kind="ExternalInput")
    bq = nc.dram_tensor("bq", [P, HG], f32, kind="ExternalInput")
    bk = nc.dram_tensor("bk", [P, HG], f32, kind="ExternalInput")
    bvm = nc.dram_tensor("bvm", [1, D], bf16, kind="ExternalInput")
    bo = nc.dram_tensor("bo", [1, D], f32, kind="ExternalInput")
    maskT = nc.dram_tensor("maskT", [P, SC], f32, kind="ExternalInput")
    out = nc.dram_tensor("out", [QB, D], f32, kind="ExternalOutput")

    with tile.TileContext(nc) as tc:
        with (
            tc.tile_pool(name="const", bufs=1) as constp,
            tc.tile_pool(name="bigx", bufs=DCH) as bigx,
            tc.tile_pool(name="bigw", bufs=DCH) as bigw,
            tc.tile_pool(name="w", bufs=4) as wpool,
            tc.tile_pool(name="kt", bufs=4) as ktpool,
            tc.tile_pool(name="va", bufs=min(SC, nc2 + 4)) as vpool,
            tc.tile_pool(name="qtz", bufs=4) as qpool,
            tc.tile_pool(name="pt", bufs=4) as ptpool,
            tc.tile_pool(name="rr", bufs=2) as rpool,
            tc.tile_pool(name="osb", bufs=2) as outp,
            tc.tile_pool(name="pp", bufs=2, space="PSUM") as pp,
            tc.tile_pool(name="psc", bufs=2, space="PSUM") as psc,
            tc.tile_pool(name="po", bufs=1, space="PSUM") as pod,
        ):
            # ---- constants
            bq_sb = constp.tile([P, HG], f32, tag="bq")
            nc.sync.dma_start(out=bq_sb[:], in_=bq[:])
            bk_sb = constp.tile([P, HG], f32, tag="bk")
            nc.sync.dma_start(out=bk_sb[:], in_=bk[:])
            mask_sb = constp.tile([P, SC], f32, tag="mask")
            nc.sync.dma_start(out=mask_sb[:], in_=maskT[:])
            bvm_sb = constp.tile([1, D], bf16, tag="bvm")
            nc.sync.dma_start(out=bvm_sb[:], in_=bvm[:])
            bo_src = constp.tile([1, D], f32, tag="bos")
            nc.sync.dma_start(out=bo_src[:], in_=bo[:])
            bo_rep = constp.tile([P, D], f32, tag="bor")
            nc.gpsimd.partition_broadcast(bo_rep[:], bo_src[:])
            ones_sb = constp.tile([1, P], bf16, tag="ones")
            nc.vector.memset(ones_sb[:], 1.0)

            for rep in range(UNROLL):
              concat = constp.tile([P, DCH, QB], bf16, tag="cat",
                                   name=f"cat{rep}")

              # ---- x^T resident in SBUF
              xq_sb = []
              for d in range(DCH):
                  t = bigx.tile([P, QB], bf16, tag="xq", name=f"xq{rep}_{d}")
                  nc.sync.dma_start(out=t[:], in_=xq[d])
                  xq_sb.append(t)
              xk_sb = []
              for d in range(DCH):
                  t = bigx.tile([P, SK], bf16, tag="xk", name=f"xk{rep}_{d}")
                  nc.sync.dma_start(out=t[:], in_=xk[d])
                  xk_sb.append(t)

              NW = 4          # waves
              for wave in range(NW):
                  groups = [2 * wave, 2 * wave + 1]
                  # ---- A: kT projection (2-head groups, [2*64 dh, s])
                  kt = []
                  for gl, g in enumerate(groups):
                      wk_t = wpool.tile([P, DCH, P], bf16, tag="wk")
                      nc.sync.dma_start(out=wk_t[:], in_=wk[g])
                      ktile = ktpool.tile([P, SK], bf16, tag="kt")
                      for sb in range(npc):
                          ps = pp.tile([P, 512], f32, tag="pp")
                          for d in range(DCH):
                              nc.tensor.matmul(
                                  ps[:, 0:256],
                                  wk_t[:, d, :],
                                  xk_sb[d][:, sb * 256:(sb + 1) * 256],
                                  start=(d == 0),
                                  stop=(d == DCH - 1),
                              )
                          nc.vector.tensor_scalar_add(
                              ktile[:, sb * 256:(sb + 1) * 256], ps[:, 0:256],
                              bk_sb[:, g:g + 1],
                          )
                      kt.append(ktile)

                  # ---- A: v projection (4 heads at once, [s, 4*64+den])
                  wv_t = wpool.tile([P, DCH, 256], bf16, tag="wv")
                  nc.sync.dma_start(out=wv_t[:], in_=wv[wave])
                  va = []
                  for sc in range(nc2):
                      vt = vpool.tile([P, 4, 65], bf16, tag="va")
                      ps = pp.tile([P, 512], f32, tag="pp",
                                   name=f"vps_{rep}_{wave}_{sc}")[:, 0:256]
                      # bias row folded into the accumulation (K=1 matmul)
                      nc.tensor.matmul(
                          ps[:],
                          ones_sb[:],
                          bvm_sb[:, wave * 256:(wave + 1) * 256],
                          start=True, stop=False,
                      )
                      for d in range(DCH):
                          nc.tensor.matmul(
                              ps[:],
                              xk_sb[d][:, sc * P:(sc + 1) * P],
                              wv_t[:, d, :],
                              start=False,
                              stop=(d == DCH - 1),
                          )
                      ps_r = ps.rearrange("p (h e) -> p h e", e=64)
                      # zero masked key rows (masked keys contribute 0 to
                      # both numerator and denominator)
                      nc.vector.tensor_scalar_mul(
                          vt[:, :, 0:64], ps_r, mask_sb[:, sc:sc + 1],
                      )
                      # denominator column = mask (1 valid, 0 pad)
                      nc.vector.tensor_scalar(
                          vt[:, :, 64:65], ps_r[:, :, 0:1], 0.0,
                          mask_sb[:, sc:sc + 1], OP.mult, OP.add,
                      )
                      va.append(vt)

                  # ---- A: q projection; per group [128 (2h x dh), 512]
                  qtz = []
                  for gl, g in enumerate(groups):
                      wq_t = wpool.tile([P, DCH, P], bf16, tag="wq")
                      nc.sync.dma_start(out=wq_t[:], in_=wq[g])
                      ps = pp.tile([P, 512], f32, tag="pp")
                      for d in range(DCH):
                          nc.tensor.matmul(
                              ps[:],
                              wq_t[:, d, :],
                              xq_sb[d][:],
                              start=(d == 0),
                              stop=(d == DCH - 1),
                          )
                      qz = qpool.tile([P, QB], bf16, tag="qtz")
                      nc.vector.tensor_scalar_add(
                          qz[:], ps[:], bq_sb[:, g:g + 1],
                      )
                      qtz.append(qz)

                  # ---- B: attention per group (2 heads row-tiled)
                  for gl, g in enumerate(groups):
                      po_t = pod.tile([65, 2, QB], f32, tag="po")
                      pts = {}

                      def emit_scores(sc):
                          # two heads concurrently: K=64 row tiles at
                          # partitions 0:64 / 64:128 of kt and qz
                          sps = psc.tile([P, 2, QB], f32, tag="ps")
                          for j in range(2):
                              lo, hi = j * 64, (j + 1) * 64
                              nc.tensor.matmul(
                                  sps[:, j, :],
                                  kt[gl][lo:hi, sc * P:(sc + 1) * P],
                                  qtz[gl][lo:hi, :],
                                  start=True,
                                  stop=True,
                              )
                          pt = ptpool.tile([P, 2, QB], bf16, tag="pt")
                          nc.scalar.activation(
                              pt[:], sps[:], AF.Exp,
                              bias=0.0, scale=0.125,
                          )
                          pts[sc] = pt

                      def emit_o(sc):
                          pt = pts.pop(sc)
                          for j in range(2):
                              nc.tensor.matmul(
                                  po_t[0:65, j, :],
                                  va[sc][:, 2 * gl + j, :],
                                  pt[:, j, :],
                                  start=(sc == 0),
                                  stop=(sc == nc2 - 1),
                              )

                      emit_scores(0)
                      if nc2 > 1:
                          emit_scores(1)
                      for sc in range(2, nc2):
                          emit_o(sc - 2)
                          emit_scores(sc)
                      if nc2 > 1:
                          emit_o(nc2 - 2)
                      emit_o(nc2 - 1)

                      # normalize: row 64 of po_t is the softmax denominator
                      den = rpool.tile([65, 2, QB], f32, tag="den")
                      nc.vector.reciprocal(den[64:65, :, :], po_t[64:65, :, :])
                      # partition_broadcast requires a base-0 input on HW
                      den0 = rpool.tile([1, 2, QB], f32, tag="den0")
                      nc.sync.dma_start(out=den0[:], in_=den[64:65, :, :])
                      rp = rpool.tile([64, 2, QB], f32, tag="rep")
                      nc.gpsimd.partition_broadcast(rp[:], den0[:])
                      nc.vector.tensor_tensor(
                          concat[0:64, g, :], po_t[0:64, 0, :],
                          rp[0:64, 0, :], OP.mult,
                      )
                      tmp = rpool.tile([64, QB], bf16, tag="tmp")
                      nc.vector.tensor_tensor(
                          tmp[:], po_t[0:64, 1, :], rp[0:64, 1, :], OP.mult,
                      )
                      nc.sync.dma_start(
                          out=concat[64:P, g, :], in_=tmp[:],
                      )

              # ---- C: output projection (contraction over h*dh in 8 chunks)
              wo_sb = []
              for c in range(DCH):
                  t = bigw.tile([P, D], bf16, tag="wo")
                  nc.sync.dma_start(out=t[:], in_=woT[c])
                  wo_sb.append(t)
              for qt_i in range(QB // P):
                  for eb in range(2):
                      ps = pp.tile([P, 512], f32, tag="pp")
                      for c in range(DCH):
                          nc.tensor.matmul(
                              ps[:],
                              concat[:, c, qt_i * P:(qt_i + 1) * P],
                              wo_sb[c][:, eb * 512:(eb + 1) * 512],
                              start=(c == 0),
                              stop=(c == DCH - 1),
                          )
                      osb = outp.tile([P, 512], f32, tag="osb")
                      nc.vector.tensor_tensor(
                          osb[:], ps[:], bo_rep[:, eb * 512:(eb + 1) * 512],
                          OP.add,
                      )
                      nc.sync.dma_start(
                          out=out[qt_i * P:(qt_i + 1) * P,
                                  eb * 512:(eb + 1) * 512],
                          in_=osb[:],
                      )

    nc.compile()
    nc.finalize()
    return nc


def prep_inputs(x, pad_mask, wq, wk, wv, bq, bk, bv, wo, bo):
    """Build per-core input maps (host-side shard + layout prep).

    Also sets the module-level NC2 (key-chunk count) that _build_program
    JIT-specializes on.
    """
    global NC2
    import ml_dtypes

    bf16 = ml_dtypes.bfloat16
    x = np.ascontiguousarray(np.asarray(x, dtype=np.float32))
    pad_mask = np.asarray(pad_mask)
    wq = np.asarray(wq, dtype=np.float32)
    wk = np.asarray(wk, dtype=np.float32)
    wv = np.asarray(wv, dtype=np.float32)
    bq = np.asarray(bq, dtype=np.float32)
    bk = np.asarray(bk, dtype=np.float32)
    bv = np.asarray(bv, dtype=np.float32)
    wo = np.asarray(wo, dtype=np.float32)
    bo = np.asarray(bo, dtype=np.float32)

    # key compaction: valid keys first (stable), per batch
    m01 = (pad_mask != 0).astype(np.float32)          # [B, S]
    order = [np.argsort(1.0 - m01[b], kind="stable") for b in range(B)]
    nv = [int(m01[b].sum()) for b in range(B)]
    nch = max(1, max(math.ceil(n / P) for n in nv))
    NC2 = min(SC, 2 * math.ceil(nch / 2))
    SK = NC2 * P

    # weights: [H, D, DH] -> per-group [g, d-part, d-chunk, out]
    def stack_groups(w, gsz):
        ws = np.ascontiguousarray(w.transpose(1, 0, 2).reshape(D, D))
        m = gsz * DH
        arr = ws.reshape(DCH, P, H // gsz, m).transpose(2, 1, 0, 3)
        return np.ascontiguousarray(arr.astype(bf16))

    wq_dev = stack_groups(wq, 2)
    wk_dev = stack_groups(wk, 2)
    wv_dev = stack_groups(wv, 4)
    woT_dev = np.ascontiguousarray(wo.T.astype(bf16)).reshape(DCH, P, D)
    bq_dev = np.ascontiguousarray(bq.reshape(HG, P).T)
    bk_dev = np.ascontiguousarray(bk.reshape(HG, P).T)
    bvm_dev = np.ascontiguousarray(bv.reshape(1, D).astype(bf16))
    bo_dev = np.ascontiguousarray(bo.reshape(1, D))

    xk_dev, maskT_dev = [], []
    for b in range(B):
        xkb = x[b][order[b]]                           # [S, D] compacted
        xkT = np.ascontiguousarray(xkb.T[:, :SK].astype(bf16))
        xk_dev.append(xkT.reshape(DCH, P, SK))
        mc = np.zeros(SC * P, np.float32)
        mc[:S] = m01[b][order[b]]
        maskT_dev.append(np.ascontiguousarray(mc.reshape(SC, P).T))

    in_maps = []
    for c in range(NCORES):
        b, qo = c // CPB, c % CPB
        xqT = np.ascontiguousarray(
            x[b][qo * QB:(qo + 1) * QB].T.astype(bf16)).reshape(DCH, P, QB)
        in_maps.append({
            "xq": xqT, "xk": xk_dev[b], "wq": wq_dev, "wk": wk_dev,
            "wv": wv_dev, "woT": woT_dev, "bq": bq_dev, "bk": bk_dev,
            "bvm": bvm_dev, "bo": bo_dev, "maskT": maskT_dev[b],
        })
    return in_maps


def kernel(**inputs):
    global LAST_RESULTS
    from concourse.bass_utils import run_bass_kernel_spmd

    in_maps = prep_inputs(**inputs)
    key = (NC2, UNROLL)
    if key not in _compiled:
        _compiled[key] = _build_program()
    nc = _compiled[key]

    res = run_bass_kernel_spmd(
        nc, in_maps, list(range(NCORES)),
        trace=bool(os.environ.get("BASS_TRACE")),
    )
    LAST_RESULTS = res

    out = np.empty((B, S, D), dtype=np.float32)
    for c in range(NCORES):
        b, qo = c // CPB, c % CPB
        out[b, qo * QB:(qo + 1) * QB, :] = res.results[c]["out"]
    return out
